# revision 13
# baseline (speedup 1.0000x reference)
"""CapsuleLikelihood kernel for Trainium2 (Bass/Tile), data-parallel over batch.

Strategy (per core = one batch element b; 8 cores):
  - diff = vote - x materialized by SWDGE DMA-accumulate onto a PE-built -x
    broadcast (rank-1 matmuls with bf16 hi/lo split of x).
  - zz = sum_d diff^2 via ACT Square + DVE segmented reduce (negated).
  - posterior logits, softmax over K+1 without max-subtraction (dummy logit
    guarantees Z >= 1e-4 and logits <= ~8).
  - routing math done per 128-n chunk in transposed (n-partition) layout via
    PE transposes; argmax via DVE max/max_index; winner via indirect DMA
    gather from HBM.
  - soft_winner = x*(1-pmp_dummy) + (sum_k E*diff)/Z + pmp_dummy*dummy_vote,
    with sum_k via PE ones-matmul over a bf16 product.
"""

import numpy as np

import bass_rust
import concourse.bass as bass
import concourse.mybir as mybir
import concourse.tile as tile
from concourse.masks import make_identity
from concourse.bass_utils import run_bass_kernel_spmd

B, K, N, D = 8, 128, 2048, 32
NT = 64            # n-tile size
NTILES = N // NT   # 32
NCH = N // 128     # 16 routing chunks
P = 128

f32 = mybir.dt.float32
bf16 = mybir.dt.bfloat16
i32 = mybir.dt.int32
u32 = mybir.dt.uint32

LOG_2PI = float(np.log(2.0 * np.pi))
DUMMY = float(-2.0 * np.log(10.0))          # -4.60517
EXP_DUMMY = 0.01                            # exp(DUMMY)
EXP_2DUMMY = 1e-4                           # exp(2*DUMMY)
EPS = 1e-16
LN_HALF = float(np.log(0.5))
BIAS_CONST = float(-0.5 * D * LOG_2PI)      # -16*log(2pi)

AL = mybir.AluOpType
AF = mybir.ActivationFunctionType


def _split_sync_waits(nc):
    """This walrus build allows only ONE sem wait per instruction; Tile emits
    more. Split extras onto preceding same-engine NoOps (engines run in
    order, so the waits still all gate the real instruction)."""
    n = 0
    for fn in nc.m.functions:
        for blk in fn.blocks:
            out = []
            changed = False
            for ins in blk.instructions:
                si = ins.sync_info
                waits = list(si.on_wait) if (si is not None and si.on_wait) else []
                if len(waits) > 1:
                    changed = True
                    for j, w in enumerate(waits[:-1]):
                        nop = mybir.InstNoOp(name=f"{ins.name}-ws{j}", ins=[], outs=[])
                        nop.engine = ins.engine
                        nop.sync_info = bass_rust.SyncInfo(on_wait=[w], on_update=[])
                        out.append(nop)
                        n += 1
                    ins.sync_info = bass_rust.SyncInfo(
                        on_wait=[waits[-1]], on_update=list(si.on_update or []))
                out.append(ins)
            if changed:
                blk.instructions = out
    return n


def _register_consts(nc, values):
    """Float biases on non-Copy activations need pre-registered const APs."""
    for v in values:
        key = (f32, float(v))
        if key in nc.const_aps.aps:
            continue
        t = nc.alloc_sbuf_tensor(f"const-f32-{v}", [128, 1], f32)
        nc.gpsimd.memset(t.ap(), float(v))
        nc.const_aps.aps[key] = t.ap()
    nc.all_engine_barrier()


def build_kernel():
    nc = bass.Bass()
    _register_consts(nc, [LN_HALF, EPS, BIAS_CONST, DUMMY, EXP_DUMMY, EXP_2DUMMY])
    x_h = nc.declare_dram_parameter("x", [N, D], f32, isOutput=False)
    vote_h = nc.declare_dram_parameter("vote", [K, N, D], f32, isOutput=False)
    scale_h = nc.declare_dram_parameter("scale", [K, N], f32, isOutput=False)
    vpp_h = nc.declare_dram_parameter("vpp", [K, N], f32, isOutput=False)
    dv_h = nc.declare_dram_parameter("dv", [N, D], f32, isOutput=False)

    lp_h = nc.declare_dram_parameter("lp_sum", [1, 1], f32, isOutput=True)
    vp_h = nc.declare_dram_parameter("vote_presence", [K, N], f32, isOutput=True)
    win_h = nc.declare_dram_parameter("winner", [N, D], f32, isOutput=True)
    wp_h = nc.declare_dram_parameter("winner_presence", [N, 1], f32, isOutput=True)
    sw_h = nc.declare_dram_parameter("soft_winner", [N, D], f32, isOutput=True)
    swp_h = nc.declare_dram_parameter("soft_winner_presence", [N, 1], f32, isOutput=True)
    pmp_h = nc.declare_dram_parameter("pmp", [N, K], f32, isOutput=True)
    mlp_h = nc.declare_dram_parameter("mlp", [K + 1, N], f32, isOutput=True)
    mxl_h = nc.declare_dram_parameter("mixing_logit", [K + 1, N], f32, isOutput=True)
    icap_h = nc.declare_dram_parameter("is_from_capsule", [N, 1], i32, isOutput=True)

    with tile.TileContext(nc) as tc:
        with (
            tc.tile_pool(name="const", bufs=1) as cpool,
            tc.tile_pool(name="batch", bufs=1) as bpool,
            tc.tile_pool(name="big", bufs=3) as big,
            tc.tile_pool(name="work", bufs=2) as work,
            tc.tile_pool(name="chk", bufs=2) as chk,
            tc.tile_pool(name="psA", bufs=1, space="PSUM") as psA,     # xb chunks
            tc.tile_pool(name="psB", bufs=1, space="PSUM") as psB,     # soft4 / lse
            tc.tile_pool(name="psT", bufs=1, space="PSUM") as psT,     # transposes
        ):
            # ---------------- constants ----------------
            ident = cpool.tile([P, P], f32, name="ident")
            make_identity(nc, ident[:])
            ones_row = cpool.tile([1, P], bf16, name="ones_row")
            nc.vector.memset(ones_row[:], 1.0)
            ones2 = cpool.tile([2, P], bf16, name="ones2")
            nc.vector.memset(ones2[:], 1.0)
            ones_col = cpool.tile([P, 1], bf16, name="ones_col")
            nc.vector.memset(ones_col[:], 1.0)
            iota_n = cpool.tile([P, NCH], i32, name="iota_n")   # 128c + p
            nc.gpsimd.iota(iota_n[:], pattern=[[128, NCH]], base=0, channel_multiplier=1)
            iota_nf = cpool.tile([P, NCH], f32, name="iota_nf")
            nc.vector.tensor_copy(iota_nf[:], iota_n[:])

            # ---------------- batch-level prep (scoped pool, freed early) ----------------
            half_is2 = bpool.tile([P, N], f32, name="half_is2")
            vpp = bpool.tile([P, N], f32, name="vpp")
            nc.sync.dma_start(vpp[:], vpp_h[:])
            bias_mix = bpool.tile([P, N], f32, name="bias_mix")
            xpair_d = nc.dram_tensor("xpair_scratch", [2, N * D], bf16)
            with tc.tile_pool(name="prep", bufs=1) as prep:
                # x as [32 tiles (partitions), NT*D], negated, bf16 hi/lo split
                x_sp = prep.tile([NTILES, NT * D], f32, name="x_sp")
                nc.sync.dma_start(x_sp[:], x_h[:].rearrange("(t a) d -> t (a d)", t=NTILES))
                xn = prep.tile([NTILES, NT * D], f32, name="xn")
                nc.vector.tensor_scalar_mul(xn[:], x_sp[:], -1.0)
                xh = prep.tile([NTILES, NT * D], bf16, name="xh")
                nc.vector.tensor_copy(xh[:], xn[:])
                xh32 = prep.tile([NTILES, NT * D], f32, name="xh32", tag="x_sp")
                nc.vector.tensor_copy(xh32[:], xh[:])
                xl32 = prep.tile([NTILES, NT * D], f32, name="xl32")
                nc.vector.tensor_tensor(xl32[:], xn[:], xh32[:], op=AL.subtract)
                xl = prep.tile([NTILES, NT * D], bf16, name="xl")
                nc.vector.tensor_copy(xl[:], xl32[:])
                # stage hi/lo via DRAM so per-tile slices land on partitions 0-1
                nc.sync.dma_start(
                    xpair_d[0:1, :].rearrange("a (t f) -> (a t) f", t=NTILES), xh[:])
                nc.sync.dma_start(
                    xpair_d[1:2, :].rearrange("a (t f) -> (a t) f", t=NTILES), xl[:])

                # scale path
                sc = prep.tile([P, N], f32, name="sc")
                nc.sync.dma_start(sc[:], scale_h[:])
                slog = prep.tile([P, N], f32, name="slog")
                nc.scalar.activation(slog[:], sc[:], AF.Ln)
                nc.scalar.activation(half_is2[:], slog[:], AF.Exp, bias=LN_HALF, scale=-2.0)

                # vpp / mixing path
                mxl = prep.tile([P, N], f32, name="mxl")
                nc.scalar.activation(mxl[:], vpp[:], AF.Ln, bias=EPS)
                nc.sync.dma_start(mxl_h[0:K, :], mxl[:])
                mxl_dummy = prep.tile([1, N], f32, name="mxl_dummy")
                nc.gpsimd.memset(mxl_dummy[:], DUMMY)
                nc.sync.dma_start(mxl_h[K:K + 1, :], mxl_dummy[:])
                vp01 = prep.tile([P, N], f32, name="vp01")
                nc.vector.tensor_scalar(vp01[:], mxl[:], DUMMY, None, op0=AL.is_gt)
                nc.sync.dma_start(vp_h[:], vp01[:])

                em = prep.tile([P, N], bf16, name="em")
                nc.scalar.activation(em[:], mxl[:], AF.Exp)
                lse_ps = psB.tile([P, N], f32, name="lse_ps", tag="soft4")
                for qq in range(4):
                    nc.tensor.matmul(lse_ps[0:1, 512 * qq:512 * qq + 512],
                                     lhsT=ones_col[:], rhs=em[:, 512 * qq:512 * qq + 512],
                                     start=True, stop=True)
                lse_row = prep.tile([1, N], f32, name="lse_row")
                nc.scalar.activation(lse_row[:], lse_ps[0:1, :], AF.Ln, bias=EXP_DUMMY)
                mlp_dummy = prep.tile([1, N], f32, name="mlp_dummy")
                nc.scalar.activation(mlp_dummy[:], lse_row[:], AF.Identity, bias=DUMMY, scale=-1.0)
                nc.sync.dma_start(mlp_h[K:K + 1, :], mlp_dummy[:])
                lse_bf = prep.tile([1, N], bf16, name="lse_bf")
                nc.vector.tensor_copy(lse_bf[:], lse_row[:])
                lseB = psB.tile([P, N], f32, name="lseB", tag="soft4")
                for qq in range(4):
                    nc.tensor.matmul(lseB[:, 512 * qq:512 * qq + 512],
                                     lhsT=ones_row[:], rhs=lse_bf[0:1, 512 * qq:512 * qq + 512],
                                     start=True, stop=True)
                mlp_sb = prep.tile([P, N], f32, name="mlp_sb", tag="vp01")
                nc.vector.tensor_tensor(mlp_sb[:], mxl[:], lseB[:], op=AL.subtract)
                nc.sync.dma_start(mlp_h[0:K, :], mlp_sb[:])

                nc.scalar.activation(bias_mix[:], slog[:], AF.Identity,
                                     bias=BIAS_CONST, scale=-float(D))
                nc.vector.tensor_tensor(bias_mix[:], bias_mix[:], mxl[:], op=AL.add)

            mlpp_sb = bpool.tile([P, NCH], f32, name="mlpp_sb")
            soft_n_by_chunk = {}
            swp_n_by_chunk = {}

            # ---------------- per-tile pipeline ----------------
            FW = NT * D + NT          # 2112: soft (64n x 32d) + swp column block
            posterior = {}
            soft4 = None
            for t in range(NTILES):
                n0 = t * NT
                dt_t = big.tile([P, NT * D], f32, name="dt_t")
                xpair_t = chk.tile([2, NT * D], bf16, name="xpair_t")
                nc.sync.dma_start(xpair_t[:], xpair_d[:, t * NT * D:(t + 1) * NT * D])
                for h in range(4):
                    fr = slice(h * 512, (h + 1) * 512)
                    xb = psA.tile([P, 512], f32, name="xb", tag="xb")
                    nc.tensor.matmul(xb[:], lhsT=ones2[:], rhs=xpair_t[:, fr],
                                     start=True, stop=True)
                    nc.scalar.copy(dt_t[:, fr], xb[:])
                # accumulate vote rows: dt_t = vote - x
                nc.gpsimd.dma_start(
                    dt_t[:], vote_h[:, n0:n0 + NT, :].rearrange("k a d -> k (a d)"),
                    accum_op=AL.add)
                # zz_neg = -sum_d diff^2
                sq_t = work.tile([P, NT * D], f32, name="sq_t")
                nc.scalar.activation(sq_t[:], dt_t[:], AF.Square)
                zzn = work.tile([P, NT], f32, name="zzn")
                nc.vector.tensor_reduce(
                    zzn[:], sq_t[:].rearrange("p (a d) -> p a d", d=D),
                    axis=mybir.AxisListType.X, op=AL.add, negate=True)
                # posterior = zz_neg*half_is2 + bias_mix
                pst = chk.tile([P, NT], f32, name="pst", bufs=4)
                nc.vector.tensor_tensor(pst[:], zzn[:], half_is2[:, n0:n0 + NT], op=AL.mult)
                nc.vector.tensor_tensor(pst[:], pst[:], bias_mix[:, n0:n0 + NT], op=AL.add)
                posterior[t] = pst
                # E in k-layout for the weighted sums
                ek = chk.tile([P, NT], f32, name="ek", bufs=4)
                nc.scalar.activation(ek[:], pst[:], AF.Exp)
                # products in bf16: E*diff (64x32) then E*vpp (64) appended
                prod = work.tile([P, FW], bf16, name="prod")
                nc.vector.tensor_tensor(
                    prod[:, 0:NT * D].rearrange("p (a d) -> p a d", d=D),
                    dt_t[:].rearrange("p (a d) -> p a d", d=D),
                    ek[:].to_broadcast([P, NT, D]), op=AL.mult)
                nc.vector.tensor_tensor(prod[:, NT * D:FW], ek[:],
                                        vpp[:, n0:n0 + NT], op=AL.mult)
                g, r = t // 4, t % 4
                if r == 0:
                    soft4 = psB.tile([P, FW], f32, name="soft4", tag="soft4")
                row = slice(32 * r, 32 * r + 1)
                tp = (0, 32 * r) if r == 3 else None
                for lo in range(0, FW, 512):
                    hi = min(lo + 512, FW)
                    nc.tensor.matmul(soft4[row, lo:hi], lhsT=ones_col[:], rhs=prod[:, lo:hi],
                                     start=True, stop=True, tile_position=tp)
                if r == 3:
                    # copy the 4 rows out (at partition bases 0/32/64/96) and
                    # reshape to [128(n), .] chunks via SBUF->SBUF DMAs
                    soft_g = work.tile([P, FW], f32, name="soft_g")
                    for a in range(4):
                        nc.scalar.copy(soft_g[32 * a:32 * a + 1, :],
                                       soft4[32 * a:32 * a + 1, :])
                    for h in range(2):
                        sn = chk.tile([P, D], f32, name="sn", bufs=4)
                        sp = chk.tile([P, 1], f32, name="sp", bufs=4)
                        for q in range(2):
                            row = 64 * h + 32 * q
                            nc.sync.dma_start(
                                sn[64 * q:64 * q + 64, :],
                                soft_g[row:row + 1, 0:NT * D].rearrange(
                                    "a (n d) -> a n d", d=D))
                            nc.sync.dma_start(
                                sp[64 * q:64 * q + 64, :],
                                soft_g[row:row + 1, NT * D:FW].rearrange(
                                    "a (n d) -> a n d", d=1))
                        soft_n_by_chunk[2 * g + h] = sn
                        swp_n_by_chunk[2 * g + h] = sp

            # ---------------- routing chunks ----------------
            for c in range(NCH):
                n0 = c * 128
                t0, t1 = 2 * c, 2 * c + 1
                eT = chk.tile([P, P], f32, name="eT")
                zacc = chk.tile([P, 1], f32, name="zacc")
                for hh, tt in ((0, t0), (1, t1)):
                    ptT = psT.tile([64, P], f32, name="ptT", tag=f"ptT{hh}")
                    nc.tensor.transpose(ptT[:], in_=posterior[tt][:], identity=ident[:])
                    nc.scalar.activation(eT[64 * hh:64 * hh + 64, :], ptT[:], AF.Exp,
                                         accum_out=zacc[64 * hh:64 * hh + 64, :])
                # mixture log prob per point, computed as ln(Z*1e4)=lnZ+ln(1e4)
                # so the Ln input sits near 1.0 where the table is accurate;
                # the host subtracts N*ln(1e4) from lp_sum.
                nc.scalar.activation(mlpp_sb[:, c:c + 1], zacc[:], AF.Ln,
                                     bias=1.0, scale=1.0e4)
                z1 = chk.tile([P, 1], f32, name="z1")
                nc.vector.tensor_scalar_add(z1[:], zacc[:], EXP_2DUMMY)
                rz = chk.tile([P, 1], f32, name="rz")
                nc.vector.reciprocal(rz[:], z1[:])
                pmpT = chk.tile([P, P], f32, name="pmpT")
                nc.scalar.mul(pmpT[:], eT[:], rz[:])
                nc.sync.dma_start(pmp_h[n0:n0 + 128, :], pmpT[:])
                # argmax over k (exp is monotonic)
                m8 = chk.tile([P, 8], f32, name="m8")
                nc.vector.max(m8[:], eT[:])
                i8 = chk.tile([P, 8], u32, name="i8")
                nc.vector.max_index(i8[:], m8[:], eT[:])
                widxf = chk.tile([P, 1], f32, name="widxf")
                nc.vector.tensor_copy(widxf[:], i8[:, 0:1])
                icap = chk.tile([P, 1], i32, name="icap")
                nc.vector.tensor_copy(icap[:], i8[:, 0:1])
                nc.sync.dma_start(icap_h[n0:n0 + 128, :], icap[:])
                # winner + winner_presence gathers: row = widx*N + n
                comb = chk.tile([P, 1], f32, name="comb")
                nc.vector.tensor_scalar_mul(comb[:], widxf[:], float(N))
                nc.vector.tensor_tensor(comb[:], comb[:], iota_nf[:, c:c + 1], op=AL.add)
                combi = chk.tile([P, 1], i32, name="combi")
                nc.vector.tensor_copy(combi[:], comb[:])
                wint = chk.tile([P, D], f32, name="wint")
                nc.gpsimd.indirect_dma_start(
                    out=wint[:], out_offset=None,
                    in_=vote_h[:].rearrange("k n d -> (k n) d"),
                    in_offset=bass.IndirectOffsetOnAxis(ap=combi[:, 0:1], axis=0))
                nc.sync.dma_start(win_h[n0:n0 + 128, :], wint[:])
                wpg = chk.tile([P, 1], f32, name="wpg")
                nc.gpsimd.indirect_dma_start(
                    out=wpg[:], out_offset=None,
                    in_=vpp_h[:].rearrange("k (n q) -> (k n) q", q=1),
                    in_offset=bass.IndirectOffsetOnAxis(ap=combi[:, 0:1], axis=0))
                nc.sync.dma_start(wp_h[n0:n0 + 128, :], wpg[:])
                # soft winner presence: (sum_k E*vpp) / Z
                swp_o = chk.tile([P, 1], f32, name="swp_o")
                nc.scalar.mul(swp_o[:], swp_n_by_chunk[c][:], rz[:])
                nc.sync.dma_start(swp_h[n0:n0 + 128, :], swp_o[:])
                # soft winner finalize
                pmpd = chk.tile([P, 1], f32, name="pmpd")
                nc.scalar.mul(pmpd[:], rz[:], EXP_2DUMMY)
                ompd = chk.tile([P, 1], f32, name="ompd")
                nc.scalar.activation(ompd[:], pmpd[:], AF.Identity, bias=1.0, scale=-1.0)
                x_n = chk.tile([P, D], f32, name="x_n")
                nc.sync.dma_start(x_n[:], x_h[n0:n0 + 128, :])
                dv_n = chk.tile([P, D], f32, name="dv_n")
                nc.sync.dma_start(dv_n[:], dv_h[n0:n0 + 128, :])
                sn = soft_n_by_chunk[c]
                f1 = chk.tile([P, D], f32, name="f1")
                nc.scalar.mul(f1[:], sn[:], rz[:])
                f2 = chk.tile([P, D], f32, name="f2")
                nc.scalar.mul(f2[:], x_n[:], ompd[:])
                nc.vector.tensor_tensor(f1[:], f1[:], f2[:], op=AL.add)
                f5 = chk.tile([P, D], f32, name="f5")
                nc.scalar.mul(f5[:], dv_n[:], pmpd[:])
                nc.vector.tensor_tensor(f1[:], f1[:], f5[:], op=AL.add)
                nc.sync.dma_start(sw_h[n0:n0 + 128, :], f1[:])

            # ---------------- log prob ----------------
            mlpp_tot = bpool.tile([P, 1], f32, name="mlpp_tot")
            nc.vector.tensor_reduce(mlpp_tot[:], mlpp_sb[:], axis=mybir.AxisListType.X, op=AL.add)
            mlpp_bf = bpool.tile([P, 1], bf16, name="mlpp_bf")
            nc.vector.tensor_copy(mlpp_bf[:], mlpp_tot[:])
            lp_ps = psT.tile([P, 1], f32, name="lp_ps", tag="ptT0")
            nc.tensor.matmul(lp_ps[0:1, :], lhsT=ones_col[:], rhs=mlpp_bf[:],
                             start=True, stop=True)
            lp_sb = bpool.tile([1, 1], f32, name="lp_sb")
            nc.scalar.copy(lp_sb[:], lp_ps[0:1, :])
            nc.sync.dma_start(lp_h[:], lp_sb[:])

    _split_sync_waits(nc)
    return nc


_NC_CACHE = None


def kernel(x, vote, scale, vote_presence_prob, dummy_vote):
    global _NC_CACHE
    if _NC_CACHE is None:
        _NC_CACHE = build_kernel()
    nc = _NC_CACHE
    dv = np.ascontiguousarray(dummy_vote[0, 0], dtype=np.float32)     # [N, D]
    in_maps = []
    for b in range(B):
        in_maps.append({
            "x": np.ascontiguousarray(x[b], dtype=np.float32),
            "vote": np.ascontiguousarray(vote[b], dtype=np.float32),
            "scale": np.ascontiguousarray(scale[b], dtype=np.float32),
            "vpp": np.ascontiguousarray(vote_presence_prob[b], dtype=np.float32),
            "dv": dv,
        })
    res = run_bass_kernel_spmd(nc, in_maps, core_ids=list(range(B)))
    R = res.results
    log_prob = np.float32(
        np.mean([R[b]["lp_sum"][0, 0] for b in range(B)]) - N * np.log(1.0e4))
    vote_presence = np.stack([R[b]["vote_presence"] for b in range(B)])
    winner = np.stack([R[b]["winner"] for b in range(B)])
    winner_presence = np.stack([R[b]["winner_presence"][:, 0] for b in range(B)])
    soft_winner = np.stack([R[b]["soft_winner"] for b in range(B)])
    soft_winner_presence = np.stack([R[b]["soft_winner_presence"][:, 0] for b in range(B)])
    pmp = np.stack([R[b]["pmp"] for b in range(B)])
    mlp = np.stack([R[b]["mlp"] for b in range(B)])
    mixing_logit = np.stack([R[b]["mixing_logit"] for b in range(B)])
    is_from_capsule = np.stack([R[b]["is_from_capsule"][:, 0] for b in range(B)]).astype(np.int32)
    return (log_prob, vote_presence, winner, winner_presence, soft_winner,
            soft_winner_presence, pmp, mlp, mixing_logit, is_from_capsule)


# revision 19
# speedup vs baseline: 1.1622x; 1.1622x over previous
"""CapsuleLikelihood kernel for Trainium2 (Bass/Tile), data-parallel over batch.

Strategy (per core = one batch element b; 8 cores):
  - diff = vote - x materialized by SWDGE DMA-accumulate onto a PE-built -x
    broadcast (rank-1 matmuls with bf16 hi/lo split of x).
  - zz = sum_d diff^2 via ACT Square + DVE segmented reduce (negated).
  - posterior logits, softmax over K+1 without max-subtraction (dummy logit
    guarantees Z >= 1e-4 and logits <= ~8).
  - routing math done per 128-n chunk in transposed (n-partition) layout via
    PE transposes; argmax via DVE max/max_index; winner via indirect DMA
    gather from HBM.
  - soft_winner = x*(1-pmp_dummy) + (sum_k E*diff)/Z + pmp_dummy*dummy_vote,
    with sum_k via PE ones-matmul over a bf16 product.
"""

import numpy as np

import bass_rust
import concourse.bass as bass
import concourse.mybir as mybir
import concourse.tile as tile
from concourse.masks import make_identity
from concourse.bass_utils import run_bass_kernel_spmd

B, K, N, D = 8, 128, 2048, 32
NT = 64            # n-tile size
NTILES = N // NT   # 32
NCH = N // 128     # 16 routing chunks
P = 128

f32 = mybir.dt.float32
bf16 = mybir.dt.bfloat16
i32 = mybir.dt.int32
u32 = mybir.dt.uint32

LOG_2PI = float(np.log(2.0 * np.pi))
DUMMY = float(-2.0 * np.log(10.0))          # -4.60517
EXP_DUMMY = 0.01                            # exp(DUMMY)
EXP_2DUMMY = 1e-4                           # exp(2*DUMMY)
EPS = 1e-16
LN_HALF = float(np.log(0.5))
BIAS_CONST = float(-0.5 * D * LOG_2PI)      # -16*log(2pi)

AL = mybir.AluOpType
AF = mybir.ActivationFunctionType


def _split_sync_waits(nc):
    """This walrus build allows only ONE sem wait per instruction; Tile emits
    more. Split extras onto preceding same-engine NoOps (engines run in
    order, so the waits still all gate the real instruction)."""
    n = 0
    for fn in nc.m.functions:
        for blk in fn.blocks:
            out = []
            changed = False
            for ins in blk.instructions:
                si = ins.sync_info
                waits = list(si.on_wait) if (si is not None and si.on_wait) else []
                if len(waits) > 1:
                    changed = True
                    for j, w in enumerate(waits[:-1]):
                        nop = mybir.InstNoOp(name=f"{ins.name}-ws{j}", ins=[], outs=[])
                        nop.engine = ins.engine
                        nop.sync_info = bass_rust.SyncInfo(on_wait=[w], on_update=[])
                        out.append(nop)
                        n += 1
                    ins.sync_info = bass_rust.SyncInfo(
                        on_wait=[waits[-1]], on_update=list(si.on_update or []))
                out.append(ins)
            if changed:
                blk.instructions = out
    return n


def _register_consts(nc, values):
    """Float biases on non-Copy activations need pre-registered const APs."""
    for v in values:
        key = (f32, float(v))
        if key in nc.const_aps.aps:
            continue
        t = nc.alloc_sbuf_tensor(f"const-f32-{v}", [128, 1], f32)
        nc.gpsimd.memset(t.ap(), float(v))
        nc.const_aps.aps[key] = t.ap()
    nc.all_engine_barrier()


def build_kernel():
    nc = bass.Bass()
    _register_consts(nc, [LN_HALF, EPS, BIAS_CONST, DUMMY, EXP_DUMMY, EXP_2DUMMY])
    x_h = nc.declare_dram_parameter("x", [N, D], f32, isOutput=False)
    vote_h = nc.declare_dram_parameter("vote", [K, N, D], f32, isOutput=False)
    scale_h = nc.declare_dram_parameter("scale", [K, N], f32, isOutput=False)
    vpp_h = nc.declare_dram_parameter("vpp", [K, N], f32, isOutput=False)
    dv_h = nc.declare_dram_parameter("dv", [N, D], f32, isOutput=False)

    lp_h = nc.declare_dram_parameter("lp_sum", [1, 1], f32, isOutput=True)
    vp_h = nc.declare_dram_parameter("vote_presence", [K, N], f32, isOutput=True)
    win_h = nc.declare_dram_parameter("winner", [N, D], f32, isOutput=True)
    wp_h = nc.declare_dram_parameter("winner_presence", [N, 1], f32, isOutput=True)
    sw_h = nc.declare_dram_parameter("soft_winner", [N, D], f32, isOutput=True)
    swp_h = nc.declare_dram_parameter("soft_winner_presence", [N, 1], f32, isOutput=True)
    pmp_h = nc.declare_dram_parameter("pmp", [N, K], f32, isOutput=True)
    mlp_h = nc.declare_dram_parameter("mlp", [K + 1, N], f32, isOutput=True)
    mxl_h = nc.declare_dram_parameter("mixing_logit", [K + 1, N], f32, isOutput=True)
    icap_h = nc.declare_dram_parameter("is_from_capsule", [N, 1], i32, isOutput=True)

    with tile.TileContext(nc) as tc:
        with (
            tc.tile_pool(name="const", bufs=1) as cpool,
            tc.tile_pool(name="batch", bufs=1) as bpool,
            tc.tile_pool(name="big", bufs=3) as big,
            tc.tile_pool(name="work", bufs=2) as work,
            tc.tile_pool(name="chk", bufs=2) as chk,
            tc.tile_pool(name="psA", bufs=1, space="PSUM") as psA,     # xb chunks
            tc.tile_pool(name="psB", bufs=1, space="PSUM") as psB,     # soft4 / lse
            tc.tile_pool(name="psT", bufs=1, space="PSUM") as psT,     # transposes
        ):
            # ---------------- constants ----------------
            ident = cpool.tile([P, P], f32, name="ident")
            make_identity(nc, ident[:])
            ones_row = cpool.tile([1, P], bf16, name="ones_row")
            nc.vector.memset(ones_row[:], 1.0)
            ones2 = cpool.tile([2, P], bf16, name="ones2")
            nc.vector.memset(ones2[:], 1.0)
            ones_col = cpool.tile([P, 1], bf16, name="ones_col")
            nc.vector.memset(ones_col[:], 1.0)
            iota_n = cpool.tile([P, NCH], i32, name="iota_n")   # 128c + p
            nc.gpsimd.iota(iota_n[:], pattern=[[128, NCH]], base=0, channel_multiplier=1)
            iota_nf = cpool.tile([P, NCH], f32, name="iota_nf")
            nc.vector.tensor_copy(iota_nf[:], iota_n[:])

            # ---------------- batch-level prep (scoped pool, freed early) ----------------
            half_is2 = bpool.tile([P, N], f32, name="half_is2")
            vpp = bpool.tile([P, N], f32, name="vpp")
            nc.sync.dma_start(vpp[:], vpp_h[:])
            bias_mix = bpool.tile([P, N], f32, name="bias_mix")
            xpair_d = nc.dram_tensor("xpair_scratch", [2, N * D], bf16)
            with tc.tile_pool(name="prep", bufs=1) as prep:
                # x as [32 tiles (partitions), NT*D], negated, bf16 hi/lo split
                x_sp = prep.tile([NTILES, NT * D], f32, name="x_sp")
                nc.sync.dma_start(x_sp[:], x_h[:].rearrange("(t a) d -> t (a d)", t=NTILES))
                xn = prep.tile([NTILES, NT * D], f32, name="xn")
                nc.vector.tensor_scalar_mul(xn[:], x_sp[:], -1.0)
                xh = prep.tile([NTILES, NT * D], bf16, name="xh")
                nc.vector.tensor_copy(xh[:], xn[:])
                xh32 = prep.tile([NTILES, NT * D], f32, name="xh32", tag="x_sp")
                nc.vector.tensor_copy(xh32[:], xh[:])
                xl32 = prep.tile([NTILES, NT * D], f32, name="xl32")
                nc.vector.tensor_tensor(xl32[:], xn[:], xh32[:], op=AL.subtract)
                xl = prep.tile([NTILES, NT * D], bf16, name="xl")
                nc.vector.tensor_copy(xl[:], xl32[:])
                # stage hi/lo via DRAM so per-tile slices land on partitions 0-1
                nc.sync.dma_start(
                    xpair_d[0:1, :].rearrange("a (t f) -> (a t) f", t=NTILES), xh[:])
                nc.sync.dma_start(
                    xpair_d[1:2, :].rearrange("a (t f) -> (a t) f", t=NTILES), xl[:])

                # scale path
                sc = prep.tile([P, N], f32, name="sc")
                nc.sync.dma_start(sc[:], scale_h[:])
                slog = prep.tile([P, N], f32, name="slog")
                nc.scalar.activation(slog[:], sc[:], AF.Ln)
                nc.scalar.activation(half_is2[:], slog[:], AF.Exp, bias=LN_HALF, scale=-2.0)

                # vpp / mixing path
                mxl = prep.tile([P, N], f32, name="mxl")
                nc.scalar.activation(mxl[:], vpp[:], AF.Ln, bias=EPS)
                nc.sync.dma_start(mxl_h[0:K, :], mxl[:])
                mxl_dummy = prep.tile([1, N], f32, name="mxl_dummy", tag="xl32")
                nc.gpsimd.memset(mxl_dummy[:], DUMMY)
                nc.sync.dma_start(mxl_h[K:K + 1, :], mxl_dummy[:])
                vp01 = prep.tile([P, N], f32, name="vp01")
                nc.vector.tensor_scalar(vp01[:], mxl[:], DUMMY, None, op0=AL.is_gt)
                nc.sync.dma_start(vp_h[:], vp01[:])

                em = prep.tile([P, N], bf16, name="em", tag="xh")
                nc.scalar.activation(em[:], mxl[:], AF.Exp)
                lse_ps = psB.tile([P, N], f32, name="lse_ps", tag="soft4")
                for qq in range(4):
                    nc.tensor.matmul(lse_ps[0:1, 512 * qq:512 * qq + 512],
                                     lhsT=ones_col[:], rhs=em[:, 512 * qq:512 * qq + 512],
                                     start=True, stop=True)
                lse_row = prep.tile([1, N], f32, name="lse_row")
                nc.scalar.activation(lse_row[:], lse_ps[0:1, :], AF.Ln, bias=EXP_DUMMY)
                mlp_dummy = prep.tile([1, N], f32, name="mlp_dummy", tag="xn")
                nc.scalar.activation(mlp_dummy[:], lse_row[:], AF.Identity, bias=DUMMY, scale=-1.0)
                nc.sync.dma_start(mlp_h[K:K + 1, :], mlp_dummy[:])
                lse_bf = prep.tile([1, N], bf16, name="lse_bf")
                nc.vector.tensor_copy(lse_bf[:], lse_row[:])
                lseB = psB.tile([P, N], f32, name="lseB", tag="soft4")
                for qq in range(4):
                    nc.tensor.matmul(lseB[:, 512 * qq:512 * qq + 512],
                                     lhsT=ones_row[:], rhs=lse_bf[0:1, 512 * qq:512 * qq + 512],
                                     start=True, stop=True)
                mlp_sb = prep.tile([P, N], f32, name="mlp_sb", tag="vp01")
                nc.vector.tensor_tensor(mlp_sb[:], mxl[:], lseB[:], op=AL.subtract)
                nc.sync.dma_start(mlp_h[0:K, :], mlp_sb[:])

                nc.scalar.activation(bias_mix[:], slog[:], AF.Identity,
                                     bias=BIAS_CONST, scale=-float(D))
                nc.vector.tensor_tensor(bias_mix[:], bias_mix[:], mxl[:], op=AL.add)

            mlpp_sb = bpool.tile([P, NCH], f32, name="mlpp_sb")
            soft_n_by_chunk = {}
            swp_n_by_chunk = {}

            # ---------------- per-tile pipeline ----------------
            FW = NT * D + NT          # 2112: soft (64n x 32d) + swp column block
            posterior = {}
            soft4 = None
            for t in range(NTILES):
                n0 = t * NT
                dt_t = big.tile([P, NT * D], f32, name="dt_t", bufs=4)
                xpair_t = chk.tile([2, NT * D], bf16, name="xpair_t")
                nc.sync.dma_start(xpair_t[:], xpair_d[:, t * NT * D:(t + 1) * NT * D])
                for h in range(2):
                    fr = slice(h * 1024, (h + 1) * 1024)
                    xb = psA.tile([P, 1024], f32, name="xb", tag="xb")
                    for qq in range(2):
                        sf = slice(qq * 512, (qq + 1) * 512)
                        nc.tensor.matmul(xb[:, sf], lhsT=ones2[:],
                                         rhs=xpair_t[:, h * 1024 + qq * 512:h * 1024 + qq * 512 + 512],
                                         start=True, stop=True)
                    nc.scalar.copy(dt_t[:, fr], xb[:])
                # accumulate vote rows: dt_t = vote - x
                nc.gpsimd.dma_start(
                    dt_t[:], vote_h[:, n0:n0 + NT, :].rearrange("k a d -> k (a d)"),
                    accum_op=AL.add)
                # zz_neg = -sum_d diff^2
                sq_t = work.tile([P, NT * D], f32, name="sq_t")
                nc.scalar.activation(sq_t[:], dt_t[:], AF.Square)
                zzn = work.tile([P, NT], f32, name="zzn")
                nc.vector.tensor_reduce(
                    zzn[:], sq_t[:].rearrange("p (a d) -> p a d", d=D),
                    axis=mybir.AxisListType.X, op=AL.add, negate=True)
                # posterior = zz_neg*half_is2 + bias_mix
                pst = chk.tile([P, NT], f32, name="pst", bufs=4)
                nc.vector.tensor_tensor(pst[:], zzn[:], half_is2[:, n0:n0 + NT], op=AL.mult)
                nc.vector.tensor_tensor(pst[:], pst[:], bias_mix[:, n0:n0 + NT], op=AL.add)
                posterior[t] = pst
                # E in k-layout for the weighted sums
                ek = chk.tile([P, NT], f32, name="ek", bufs=4)
                nc.scalar.activation(ek[:], pst[:], AF.Exp)
                # products in bf16: E*diff (64x32) then E*vpp (64) appended
                prod = work.tile([P, FW], bf16, name="prod")
                nc.vector.tensor_tensor(
                    prod[:, 0:NT * D].rearrange("p (a d) -> p a d", d=D),
                    dt_t[:].rearrange("p (a d) -> p a d", d=D),
                    ek[:].to_broadcast([P, NT, D]), op=AL.mult)
                nc.vector.tensor_tensor(prod[:, NT * D:FW], ek[:],
                                        vpp[:, n0:n0 + NT], op=AL.mult)
                g, r = t // 4, t % 4
                if r == 0:
                    soft4 = psB.tile([P, FW], f32, name="soft4", tag="soft4")
                row = slice(32 * r, 32 * r + 1)
                tp = (0, 32 * r) if r == 3 else None
                for lo in range(0, FW, 512):
                    hi = min(lo + 512, FW)
                    nc.tensor.matmul(soft4[row, lo:hi], lhsT=ones_col[:], rhs=prod[:, lo:hi],
                                     start=True, stop=True, tile_position=tp)
                if r == 3:
                    # copy the 4 rows out (at partition bases 0/32/64/96) and
                    # reshape to [128(n), .] chunks via SBUF->SBUF DMAs
                    soft_g = work.tile([P, FW], f32, name="soft_g")
                    for a in range(4):
                        nc.scalar.copy(soft_g[32 * a:32 * a + 1, :],
                                       soft4[32 * a:32 * a + 1, :])
                    for h in range(2):
                        sn = chk.tile([P, D], f32, name="sn", bufs=4)
                        sp = chk.tile([P, 1], f32, name="sp", bufs=4)
                        for q in range(2):
                            row = 64 * h + 32 * q
                            nc.sync.dma_start(
                                sn[64 * q:64 * q + 64, :],
                                soft_g[row:row + 1, 0:NT * D].rearrange(
                                    "a (n d) -> a n d", d=D))
                            nc.sync.dma_start(
                                sp[64 * q:64 * q + 64, :],
                                soft_g[row:row + 1, NT * D:FW].rearrange(
                                    "a (n d) -> a n d", d=1))
                        soft_n_by_chunk[2 * g + h] = sn
                        swp_n_by_chunk[2 * g + h] = sp

            # ---------------- routing chunks ----------------
            for c in range(NCH):
                n0 = c * 128
                t0, t1 = 2 * c, 2 * c + 1
                eT = chk.tile([P, P], f32, name="eT")
                zacc = chk.tile([P, 1], f32, name="zacc")
                for hh, tt in ((0, t0), (1, t1)):
                    ptT = psT.tile([64, P], f32, name="ptT", tag="ptT")
                    nc.tensor.transpose(ptT[:], in_=posterior[tt][:], identity=ident[:])
                    nc.scalar.activation(eT[64 * hh:64 * hh + 64, :], ptT[:], AF.Exp,
                                         accum_out=zacc[64 * hh:64 * hh + 64, :])
                # mixture log prob per point, computed as ln(Z*1e4)=lnZ+ln(1e4)
                # so the Ln input sits near 1.0 where the table is accurate;
                # the host subtracts N*ln(1e4) from lp_sum.
                nc.scalar.activation(mlpp_sb[:, c:c + 1], zacc[:], AF.Ln,
                                     bias=1.0, scale=1.0e4)
                z1 = chk.tile([P, 1], f32, name="z1")
                nc.vector.tensor_scalar_add(z1[:], zacc[:], EXP_2DUMMY)
                rz = chk.tile([P, 1], f32, name="rz")
                nc.vector.reciprocal(rz[:], z1[:])
                pmpT = chk.tile([P, P], f32, name="pmpT")
                nc.scalar.mul(pmpT[:], eT[:], rz[:])
                nc.sync.dma_start(pmp_h[n0:n0 + 128, :], pmpT[:])
                # argmax over k (exp is monotonic)
                m8 = chk.tile([P, 8], f32, name="m8")
                nc.vector.max(m8[:], eT[:])
                i8 = chk.tile([P, 8], u32, name="i8")
                nc.vector.max_index(i8[:], m8[:], eT[:])
                widxf = chk.tile([P, 1], f32, name="widxf")
                nc.vector.tensor_copy(widxf[:], i8[:, 0:1])
                icap = chk.tile([P, 1], i32, name="icap")
                nc.vector.tensor_copy(icap[:], i8[:, 0:1])
                nc.sync.dma_start(icap_h[n0:n0 + 128, :], icap[:])
                # winner + winner_presence gathers: row = widx*N + n
                comb = chk.tile([P, 1], f32, name="comb")
                nc.vector.tensor_scalar_mul(comb[:], widxf[:], float(N))
                nc.vector.tensor_tensor(comb[:], comb[:], iota_nf[:, c:c + 1], op=AL.add)
                combi = chk.tile([P, 1], i32, name="combi")
                nc.vector.tensor_copy(combi[:], comb[:])
                wint = chk.tile([P, D], f32, name="wint")
                nc.gpsimd.indirect_dma_start(
                    out=wint[:], out_offset=None,
                    in_=vote_h[:].rearrange("k n d -> (k n) d"),
                    in_offset=bass.IndirectOffsetOnAxis(ap=combi[:, 0:1], axis=0))
                nc.sync.dma_start(win_h[n0:n0 + 128, :], wint[:])
                wpg = chk.tile([P, 1], f32, name="wpg")
                nc.gpsimd.indirect_dma_start(
                    out=wpg[:], out_offset=None,
                    in_=vpp_h[:].rearrange("k (n q) -> (k n) q", q=1),
                    in_offset=bass.IndirectOffsetOnAxis(ap=combi[:, 0:1], axis=0))
                nc.sync.dma_start(wp_h[n0:n0 + 128, :], wpg[:])
                # soft winner presence: (sum_k E*vpp) / Z
                swp_o = chk.tile([P, 1], f32, name="swp_o")
                nc.scalar.mul(swp_o[:], swp_n_by_chunk[c][:], rz[:])
                nc.sync.dma_start(swp_h[n0:n0 + 128, :], swp_o[:])
                # soft winner finalize
                pmpd = chk.tile([P, 1], f32, name="pmpd")
                nc.scalar.mul(pmpd[:], rz[:], EXP_2DUMMY)
                ompd = chk.tile([P, 1], f32, name="ompd")
                nc.scalar.activation(ompd[:], pmpd[:], AF.Identity, bias=1.0, scale=-1.0)
                x_n = chk.tile([P, D], f32, name="x_n")
                nc.sync.dma_start(x_n[:], x_h[n0:n0 + 128, :])
                dv_n = chk.tile([P, D], f32, name="dv_n")
                nc.sync.dma_start(dv_n[:], dv_h[n0:n0 + 128, :])
                sn = soft_n_by_chunk[c]
                f1 = chk.tile([P, D], f32, name="f1")
                nc.scalar.mul(f1[:], sn[:], rz[:])
                f2 = chk.tile([P, D], f32, name="f2")
                nc.scalar.mul(f2[:], x_n[:], ompd[:])
                nc.vector.tensor_tensor(f1[:], f1[:], f2[:], op=AL.add)
                f5 = chk.tile([P, D], f32, name="f5")
                nc.scalar.mul(f5[:], dv_n[:], pmpd[:])
                nc.vector.tensor_tensor(f1[:], f1[:], f5[:], op=AL.add)
                nc.sync.dma_start(sw_h[n0:n0 + 128, :], f1[:])

            # ---------------- log prob ----------------
            mlpp_tot = bpool.tile([P, 1], f32, name="mlpp_tot")
            nc.vector.tensor_reduce(mlpp_tot[:], mlpp_sb[:], axis=mybir.AxisListType.X, op=AL.add)
            mlpp_bf = bpool.tile([P, 1], bf16, name="mlpp_bf")
            nc.vector.tensor_copy(mlpp_bf[:], mlpp_tot[:])
            lp_ps = psT.tile([P, 1], f32, name="lp_ps", tag="ptT")
            nc.tensor.matmul(lp_ps[0:1, :], lhsT=ones_col[:], rhs=mlpp_bf[:],
                             start=True, stop=True)
            lp_sb = bpool.tile([1, 1], f32, name="lp_sb")
            nc.scalar.copy(lp_sb[:], lp_ps[0:1, :])
            nc.sync.dma_start(lp_h[:], lp_sb[:])

    _split_sync_waits(nc)
    return nc


_NC_CACHE = None


def kernel(x, vote, scale, vote_presence_prob, dummy_vote):
    global _NC_CACHE
    if _NC_CACHE is None:
        _NC_CACHE = build_kernel()
    nc = _NC_CACHE
    dv = np.ascontiguousarray(dummy_vote[0, 0], dtype=np.float32)     # [N, D]
    in_maps = []
    for b in range(B):
        in_maps.append({
            "x": np.ascontiguousarray(x[b], dtype=np.float32),
            "vote": np.ascontiguousarray(vote[b], dtype=np.float32),
            "scale": np.ascontiguousarray(scale[b], dtype=np.float32),
            "vpp": np.ascontiguousarray(vote_presence_prob[b], dtype=np.float32),
            "dv": dv,
        })
    res = run_bass_kernel_spmd(nc, in_maps, core_ids=list(range(B)))
    R = res.results
    log_prob = np.float32(
        np.mean([R[b]["lp_sum"][0, 0] for b in range(B)]) - N * np.log(1.0e4))
    vote_presence = np.stack([R[b]["vote_presence"] for b in range(B)])
    winner = np.stack([R[b]["winner"] for b in range(B)])
    winner_presence = np.stack([R[b]["winner_presence"][:, 0] for b in range(B)])
    soft_winner = np.stack([R[b]["soft_winner"] for b in range(B)])
    soft_winner_presence = np.stack([R[b]["soft_winner_presence"][:, 0] for b in range(B)])
    pmp = np.stack([R[b]["pmp"] for b in range(B)])
    mlp = np.stack([R[b]["mlp"] for b in range(B)])
    mixing_logit = np.stack([R[b]["mixing_logit"] for b in range(B)])
    is_from_capsule = np.stack([R[b]["is_from_capsule"][:, 0] for b in range(B)]).astype(np.int32)
    return (log_prob, vote_presence, winner, winner_presence, soft_winner,
            soft_winner_presence, pmp, mlp, mixing_logit, is_from_capsule)
